# revision 1
# baseline (speedup 1.0000x reference)
"""Trainium2 Bass kernel for nn_LocationEffect (GAT + temporal sigmoid attention).

out[s2*N+b, t1*N+a] = sw[b, t1, s2] * adj[b, a]
where sw = sigmoid(scale * nf nf^T per node), nf = GAT(raw_features, adj).

Sharding: row-shard the [12000, 12000] output over the node dim b.
Each of the 8 cores owns B = N/8 = 125 nodes: it computes the GAT for its
125 query rows (keys/values = all 1000 nodes, replicated), its sw slice
[125, 12, 12], and writes a [12, 125, 12000] output slab.

Precision strategy (tolerance is 2e-2): inputs are fed to the device as
bf16 (features) / f16 (mask), the GAT attention runs in bf16 on the PE
(1 cycle/row vs 4 for fp32 matmuls), softmax statistics and node features
stay f32, and the output is written in float16, which halves HBM write
traffic: 36 MB/core -> ~100 us at the 360 GB/s cost-model roofline.
Blocks are computed on DVE (f16 hits the 4x DVE mode) and written as two
batched DMAs per timestamp t: the new row tile s2 = t (columns t1 <= t)
and the new column t1 = t of earlier row tiles (s2 < t), so output DMA
starts while the GAT is still running and hides it.
"""

import sys

import numpy as np

if "/opt/trn_rl_repo" not in sys.path:
    sys.path.insert(0, "/opt/trn_rl_repo")

T, N, D = 12, 1000, 64
NCORES = 8
B = N // NCORES  # 125 nodes per core
C = 8  # n-chunks of size B for K-tiling / transposes
NPAIR = T * (T + 1) // 2  # symmetric (t1 <= s2) pairs
NWARM = 24  # PE warmup transposes (p-state ramp)

_CACHE = {}


def pidx(a, b):
    """s2-major triangular index of the unordered pair {a, b}."""
    lo, hi = min(a, b), max(a, b)
    return hi * (hi + 1) // 2 + lo


def _build(repeat=1, parts="all"):
    import concourse.bacc as bacc
    import concourse.mybir as mybir
    import concourse.tile as tile
    from concourse.masks import make_identity

    f32 = mybir.dt.float32
    f16 = mybir.dt.float16
    bf16 = mybir.dt.bfloat16
    Act = mybir.ActivationFunctionType
    Alu = mybir.AluOpType

    nc = bacc.Bacc(
        "TRN2",
        target_bir_lowering=False,
        debug=False,
        enable_asserts=False,
        num_devices=NCORES,
    )
    rf = nc.dram_tensor("rf", (T, N, D), bf16, kind="ExternalInput").ap()
    rfq = nc.dram_tensor("rfq", (B, T, D), bf16, kind="ExternalInput").ap()
    adj = nc.dram_tensor("adj", (B, N), f16, kind="ExternalInput").ap()
    out = nc.dram_tensor("out", (T, B, T * N), f16, kind="ExternalOutput").ap()
    # b-major view for the strided column-slab DMA
    out_bsm = out.rearrange("s b m -> b s m")

    with tile.TileContext(nc) as tc:
        with (
            tc.tile_pool(name="const", bufs=1) as consts,
            tc.tile_pool(name="main", bufs=1) as main,
            tc.tile_pool(name="ktp", bufs=2) as ktp,
            tc.tile_pool(name="expp", bufs=2) as expp,
            tc.tile_pool(name="swp", bufs=2) as swp,
            tc.tile_pool(name="rowp", bufs=2) as rowp,
            tc.tile_pool(name="colp", bufs=2) as colp,
            tc.tile_pool(name="ps1", bufs=1, space="PSUM") as ps1,
            tc.tile_pool(name="ps2", bufs=2, space="PSUM") as ps2,
        ):
            ident = consts.tile([128, 128], bf16)
            make_identity(nc, ident[:])

            # inputs, in first-use order: queries, h[t=0], mask on the Act
            # queue (3 dispatches, ~2 us, then Act is free for GAT t0);
            # h[t>=1] through the idle Pool engine's SWDGE path so the Act
            # sequencer never queues behind 11 DMA dispatches.
            rfq_sb = main.tile([B, T, D], bf16)
            nc.scalar.dma_start(out=rfq_sb[:], in_=rfq)
            # h with interleaved node chunks: chunk c = nodes {p*C + c}, so
            # partition p holds rows p*C..p*C+7 of rf[t] — 1 KB contiguous
            # per (t, p) descriptor.
            h_nat = main.tile([B, T, C, D], bf16)
            rf_r = rf.rearrange("t (p c) d -> p t c d", p=B)
            nc.scalar.dma_start(out=h_nat[:, 0, :, :], in_=rf_r[:, 0, :, :])
            adjh = main.tile([B, N], f16)
            nc.scalar.dma_start(out=adjh[:], in_=adj)
            for t in range(1, T):
                nc.gpsimd.dma_start(out=h_nat[:, t, :, :], in_=rf_r[:, t, :, :])

            # mask in the interleaved m-order used by h_nat chunks:
            # adjp[b, c*B + p] = adjh[b, p*C + c]
            adjp = main.tile([B, N], bf16)
            nc.vector.tensor_copy(
                adjp[:].rearrange("b (c p) -> b c p", c=C),
                adjh[:].rearrange("b (p c) -> b c p", c=C),
            )

            nf = main.tile([B, T, D], f32)  # normalized node features
            den = main.tile([B, T], f32)
            invden = main.tile([B, T], f32)
            swdot = main.tile([B, NPAIR], f32)
            swth = main.tile([B, NPAIR], f32)
            sw = main.tile([B, NPAIR], f32)

            for _rep in range(repeat):
                for t in range(T if parts != "p4" else 0):
                    # ---- GAT timestamp t ----
                    kt_ps = ps1.tile([64, C, 128], bf16, name="kt_ps")  # 1 bank
                    for c in range(C):
                        nc.tensor.transpose(
                            kt_ps[:, c, 0:B], h_nat[:, t, c, :], ident[0:B, 0:B]
                        )
                    q_ps = ps1.tile([64, 128], bf16, name="q_ps")  # 1 bank
                    nc.tensor.transpose(q_ps[:, 0:B], rfq_sb[:, t, :], ident[0:B, 0:B])
                    keysT = ktp.tile([64, C, B], bf16, name="keysT")
                    nc.scalar.copy(keysT[:], kt_ps[:, :, 0:B])
                    qT = ktp.tile([64, B], bf16, name="qT")
                    nc.scalar.copy(qT[:], q_ps[:, 0:B])

                    keysT_flat = keysT[:].rearrange("d c p -> d (c p)")
                    # raw scores -> exp(0.125 * scores). No max-subtraction:
                    # scaled scores <= ~15 for these inputs, exp stays finite
                    # and the softmax ratio is shift-invariant.
                    exps = expp.tile([B, N], bf16, name="exps")
                    for half in range(2):
                        sc_ps = ps2.tile([B, 512], f32, name="sc_ps")  # 1 bank x2
                        nc.tensor.matmul(
                            sc_ps[:, 0:500],
                            qT[:],
                            keysT_flat[:, half * 500 : (half + 1) * 500],
                            start=True,
                            stop=True,
                        )
                        nc.scalar.activation(
                            exps[:, half * 500 : (half + 1) * 500],
                            sc_ps[:, 0:500],
                            Act.Exp,
                            scale=0.125,
                        )
                    # masked exp (adj gate): plain tensor-tensor mult hits the
                    # 4x DVE mode (all-SBUF 2-byte operands). The softmax
                    # denominator comes from the ones-column matmul below.
                    mexp = expp.tile([B, N], bf16, name="mexp")
                    nc.vector.tensor_tensor(
                        out=mexp[:], in0=exps[:], in1=adjp[:], op=Alu.mult
                    )
                    # attn^T chunks via PE transposes
                    at_ps = ps1.tile([B, C, 128], bf16, name="at_ps")  # 1 bank
                    for c in range(C):
                        nc.tensor.transpose(
                            at_ps[:, c, 0:B],
                            mexp[:, c * B : (c + 1) * B],
                            ident[0:B, 0:B],
                        )
                    attnT = ktp.tile([B, C, B], bf16, name="attnT")
                    nc.scalar.copy(attnT[:], at_ps[:, :, 0:B])
                    # softmax denominator: fast-mode DVE reduce over mexp
                    nc.vector.tensor_reduce(
                        out=den[:, t : t + 1],
                        in_=mexp[:],
                        axis=mybir.AxisListType.X,
                        op=Alu.add,
                    )
                    # nf_unnorm = attn^T.T @ h, K-accumulated over 8 chunks
                    nf_ps = ps1.tile([B, 64], f32, name="nf_ps")  # 1 bank
                    for c in range(C):
                        nc.tensor.matmul(
                            nf_ps[:],
                            attnT[:, c, :],
                            h_nat[:, t, c, :],
                            start=(c == 0),
                            stop=(c == C - 1),
                        )
                    nc.vector.reciprocal(invden[:, t : t + 1], den[:, t : t + 1])
                    # normalize while moving PSUM -> SBUF
                    nc.vector.tensor_scalar_mul(
                        nf[:, t, :], nf_ps[:], invden[:, t : t + 1]
                    )

                    # ---- sw pairs {t1 <= t, t}: ready now ----
                    seg = t * (t + 1) // 2
                    for t1 in range(t + 1):
                        prod = swp.tile([B, D], f32, name="prod")
                        nc.vector.scalar_tensor_tensor(
                            out=prod[:],
                            in0=nf[:, t1, :],
                            scalar=1.0,
                            in1=nf[:, t, :],
                            op0=Alu.mult,
                            op1=Alu.mult,
                            accum_out=swdot[:, seg + t1 : seg + t1 + 1],
                        )
                    # sigmoid(x) = 0.5*(1 + tanh(x/2)): tanh shares the Act
                    # function table with exp/copy, so the engine never
                    # reloads tables (1283 ns each) mid-kernel.
                    nc.scalar.activation(
                        swth[:, seg : seg + t + 1],
                        swdot[:, seg : seg + t + 1],
                        Act.Tanh,
                        scale=0.0625,
                    )
                    nc.vector.tensor_scalar(
                        out=sw[:, seg : seg + t + 1],
                        in0=swth[:, seg : seg + t + 1],
                        scalar1=0.5,
                        scalar2=0.5,
                        op0=Alu.mult,
                        op1=Alu.add,
                    )

                    # ---- output blocks unlocked by timestamp t ----
                    if parts != "gat":
                        # row tile s2 = t: columns t1 = 0..t, one batched DMA
                        rows = rowp.tile([B, t + 1, N], f16, name="rows")
                        for t1 in range(t + 1):
                            col = pidx(t1, t)
                            nc.vector.tensor_scalar_mul(
                                rows[:, t1, :], adjh[:], sw[:, col : col + 1]
                            )
                        nc.sync.dma_start(
                            out=out[t, :, 0 : (t + 1) * N],
                            in_=rows[:].rearrange("b t n -> b (t n)"),
                        )
                        # column t1 = t of earlier row tiles s2 = 0..t-1,
                        # one strided DMA
                        if t > 0:
                            cols = colp.tile([B, t, N], f16, name="cols")
                            for s2 in range(t):
                                col = pidx(t, s2)
                                nc.vector.tensor_scalar_mul(
                                    cols[:, s2, :], adjh[:], sw[:, col : col + 1]
                                )
                            nc.sync.dma_start(
                                out=out_bsm[:, 0:t, t * N : (t + 1) * N],
                                in_=cols[:],
                            )

    nc.compile()
    return nc


def _get_nc(repeat=1, parts="all"):
    key = ("nc", repeat, parts)
    if key not in _CACHE:
        _CACHE[key] = _build(repeat, parts)
    return _CACHE[key]


def make_in_maps(rf_f32, adj_i32):
    """Per-core input dicts from the full f32/i32 host arrays."""
    import ml_dtypes

    bf16 = ml_dtypes.bfloat16
    rf16 = np.ascontiguousarray(np.asarray(rf_f32, dtype=np.float32)).astype(bf16)
    adjh = np.asarray(adj_i32).astype(np.float16)
    in_maps = []
    for k in range(NCORES):
        sl = slice(k * B, (k + 1) * B)
        in_maps.append(
            {
                "rf": rf16,
                "rfq": np.ascontiguousarray(rf16[:, sl, :].transpose(1, 0, 2)),
                "adj": np.ascontiguousarray(adjh[sl, :]),
            }
        )
    return in_maps


def kernel(raw_features, adj):
    from concourse.bass_utils import run_bass_kernel_spmd

    nc = _get_nc()
    in_maps = make_in_maps(raw_features, adj)
    res = run_bass_kernel_spmd(nc, in_maps, core_ids=list(range(NCORES)))
    out = np.empty((T * N, T * N), dtype=np.float32)
    ov = out.reshape(T, NCORES, B, T * N)
    for k in range(NCORES):
        ov[:, k] = np.asarray(res.results[k]["out"], dtype=np.float32).reshape(
            T, B, T * N
        )
    return out



# revision 5
# speedup vs baseline: 1.2397x; 1.2397x over previous
"""Trainium2 Bass kernel for nn_LocationEffect (GAT + temporal sigmoid attention).

out[s2*N+b, t1*N+a] = sw[b, t1, s2] * adj[b, a]
where sw = sigmoid(scale * nf nf^T per node), nf = GAT(raw_features, adj).

Sharding: row-shard the [12000, 12000] output over the node dim b.
Each of the 8 cores owns B = N/8 = 125 nodes: it computes the GAT for its
125 query rows (keys/values = all 1000 nodes, replicated), its sw slice
[125, 12, 12], and writes a [12, 125, 12000] output slab.

This version restructures the GAT so the per-step compute fits well under
the DMA roofline and mixes two output encodings to cut HBM write traffic:

- Scores are computed TRANSPOSED (scT[m, b] = k_m . q_b) so the masked
  exp weights come out of the Activation engine already in the [key-part,
  query] layout the nf matmul wants: no per-step PE transposes of the
  attention row and no PSUM->SBUF copy for it.
- The softmax denominator comes from a ones-column appended to h (column
  64 of each value chunk), so it falls out of the nf matmul for free.
- sigmoid is computed as 1/(1+exp(-x)) (Act exp + DVE reciprocal), and
  the u8 log-encode ln(sw) = -ln(1+exp(-x)) reuses the same intermediate:
  exp, ln, copy, identity all live in ONE act function table set
  (natural_log_exp_and_others), so the table is loaded exactly once.
- Output blocks are written either as f16 (2B) or as a log-encoded uint8
  (1B): q = round(BQ - AQ*ln(1/sw)); host decodes via a 256-entry LUT.
  Max elementwise rel err of the u8 encode is ~1.1e-2 and its norm-rel
  contribution ~6e-3, both far inside the 2e-2 gate. A small static
  scheduler picks, per GAT step, how many blocks go u8 vs f16 and which
  engine (DVE 4x-mode f16 = 321ns, Act copy-with-scale = 1018ns, Pool
  tensor-scalar = 1484ns, DVE u8 = 1102ns) computes each, balancing
  per-step engine busy time against the shrinking DMA time.
- All inputs are host-prepacked so each load is one HWDGE DMA with
  per-partition-contiguous descriptors (no SWDGE prep serialization).
"""

import math
import sys

import numpy as np

if "/opt/trn_rl_repo" not in sys.path:
    sys.path.insert(0, "/opt/trn_rl_repo")

T, N, D = 12, 1000, 64
NCORES = 8
B = N // NCORES  # 125 nodes per core
C = 8  # n-chunks of size B for K-tiling
DP = D + 1  # +1 ones column for the softmax denominator
NPAIR = T * (T + 1) // 2  # symmetric (t1 <= s2) pairs

U8_ENABLE = True
SW_FLOOR = 0.004  # smallest sw the u8 log code can represent
AQ = 254.0 / math.log(1.0 / SW_FLOOR)  # 46.005...
BQ = 255.0

_CACHE = {}


def pidx(a, b):
    """s2-major triangular index of the unordered pair {a, b}."""
    lo, hi = min(a, b), max(a, b)
    return hi * (hi + 1) // 2 + lo


def u8_lut():
    """Decode table: LUT[q] = sigmoid value encoded by code q (0 -> 0.0)."""
    q = np.arange(256, dtype=np.float64)
    lut = np.exp((q - BQ) / AQ)
    lut[0] = 0.0
    return lut.astype(np.float32)


def _schedule():
    """Per-step block plan.

    For GAT step t the kernel emits the new row tile (s2=t, t1=0..t) and
    the new column tile (t1=t, s2=0..t-1).  Each block is 125x1000 and can
    be written f16 (DVE fast mode) or u8 (any engine).  Choose per step:
      kr[t]: row blocks t1 <  kr are f16, t1 >= kr are u8
      kc[t]: col blocks s2 <  kc are f16, s2 >= kc are u8
    plus an engine for every u8 block and for the keysT PSUM->SBUF copy,
    minimizing the per-step makespan (engine busy vs DMA time).
    Returns list of dicts per t.
    """
    DVE_F16 = 0.321
    DVE_U8 = 1.102
    ACT_BLK = 1.018
    POOL_BLK = 1.484
    KC_COST = {"dve": 0.647, "act": 1.018, "pool": 1.484}
    sched = []
    for t in range(T):
        n = 2 * t + 1
        gat_dve = 1.08 + 0.13 * t
        gat_act = 1.60
        has_kc = t + 1 < T
        best = None
        for u in range(0, (n + 1) if U8_ENABLE else 1):
            f = n - u
            tdma = 0.694 * f + 0.347 * u
            for a in range(u + 1):
                for p in range(u - a + 1):
                    d = u - a - p
                    # Pool/GPSIMD cannot read PSUM, so the keysT copy is
                    # DVE or Act only.
                    for kce in ("dve", "act") if has_kc else ("none",):
                        tdve = gat_dve + DVE_F16 * f + DVE_U8 * d
                        tact = gat_act + ACT_BLK * a
                        tpool = POOL_BLK * p
                        if kce == "dve":
                            tdve += KC_COST["dve"]
                        elif kce == "act":
                            tact += KC_COST["act"]
                        elif kce == "pool":
                            tpool += KC_COST["pool"]
                        mk = max(tdve, tact, tpool, tdma)
                        # minimize makespan; tie-break: more u8 (less DMA),
                        # then lighter DVE (shorter critical path)
                        key = (round(mk * 16), -u, round(tdve * 16))
                        if best is None or key < best[0]:
                            best = (key, u, a, p, d, kce)
        _, u, a, p, d, kce = best
        f = n - u
        # distribute f16 blocks / u8 blocks over the concrete (row, col)
        # slots.  Rows tile has t+1 slots, cols tile t slots.  Put the f16
        # blocks at the low-t1 / low-s2 end (any split works; ranges must
        # be contiguous).  Rows first.
        fr = min(f, t + 1)
        fc = f - fr
        kr = fr  # rows t1 < kr f16
        kc = fc  # cols s2 < kc f16
        # engine list for u8 blocks in emission order (rows u8 then cols u8)
        engines = ["act"] * a + ["pool"] * p + ["dve"] * d
        # interleave so no engine gets all-consecutive blocks
        engines.sort()
        inter = []
        while engines:
            for e in ("act", "pool", "dve"):
                if e in engines:
                    engines.remove(e)
                    inter.append(e)
        sched.append(
            {"kr": kr, "kc": kc, "u8_eng": inter, "kc_eng": kce if has_kc else None}
        )
    return sched


SCHED = _schedule()


def _build():
    import concourse.bacc as bacc
    import concourse.mybir as mybir
    import concourse.tile as tile
    from concourse.masks import make_identity

    f32 = mybir.dt.float32
    f16 = mybir.dt.float16
    bf16 = mybir.dt.bfloat16
    u8 = mybir.dt.uint8
    Act = mybir.ActivationFunctionType
    Alu = mybir.AluOpType

    nc = bacc.Bacc(
        "TRN2",
        target_bir_lowering=False,
        debug=False,
        enable_asserts=False,
        num_devices=NCORES,
    )
    qT = nc.dram_tensor("qT", (D, T, B), bf16, kind="ExternalInput").ap()
    hT0 = nc.dram_tensor("hT0", (D, C, B), bf16, kind="ExternalInput").ap()
    hn = nc.dram_tensor("hn", (B, T, C, DP), bf16, kind="ExternalInput").ap()
    adjT = nc.dram_tensor("adjT", (B, C, B), bf16, kind="ExternalInput").ap()
    adjh = nc.dram_tensor("adjh", (B, N), f16, kind="ExternalInput").ap()
    out16 = nc.dram_tensor("out16", (T, B, T * N), f16, kind="ExternalOutput").ap()
    out16_bsm = out16.rearrange("s b m -> b s m")
    out8 = nc.dram_tensor("out8", (T, B, T * N), u8, kind="ExternalOutput").ap()
    out8_bsm = out8.rearrange("s b m -> b s m")

    # max tile sizes for the block pools, from the schedule
    max_r16 = max(SCHED[t]["kr"] for t in range(T))
    max_r8 = max(t + 1 - SCHED[t]["kr"] for t in range(T))
    max_c16 = max(SCHED[t]["kc"] for t in range(T))
    max_c8 = max(t - SCHED[t]["kc"] for t in range(T))

    with tile.TileContext(nc) as tc:
        with (
            tc.tile_pool(name="const", bufs=1) as consts,
            tc.tile_pool(name="main", bufs=1) as main,
            tc.tile_pool(name="expp", bufs=2) as expp,
            tc.tile_pool(name="mexpp", bufs=2) as mexpp,
            tc.tile_pool(name="prodp", bufs=2) as prodp,
            tc.tile_pool(name="r16p", bufs=2) as r16p,
            tc.tile_pool(name="r8p", bufs=2) as r8p,
            tc.tile_pool(name="c16p", bufs=2) as c16p,
            tc.tile_pool(name="c8p", bufs=2) as c8p,
            tc.tile_pool(name="ps_kt", bufs=2, space="PSUM") as ps_kt,
            tc.tile_pool(name="ps_sc", bufs=2, space="PSUM") as ps_sc,
            tc.tile_pool(name="ps_nf", bufs=2, space="PSUM") as ps_nf,
        ):
            ident = consts.tile([128, 128], bf16)
            make_identity(nc, ident[:])

            # ---- inputs: critical pieces first, all single HWDGE DMAs ----
            qT_sb = main.tile([D, T, B], bf16)
            nc.sync.dma_start(out=qT_sb[:], in_=qT)
            keysT = main.tile([D, T, C, B], bf16)
            nc.sync.dma_start(out=keysT[:, 0], in_=hT0)
            adjT_sb = main.tile([B, C, B], bf16)
            nc.sync.dma_start(out=adjT_sb[:], in_=adjT)
            hn_sb = main.tile([B, T, C, DP], bf16)
            nc.sync.dma_start(out=hn_sb[:, 0:3], in_=hn[:, 0:3])
            adjh_sb = main.tile([B, N], f16)
            nc.sync.dma_start(out=adjh_sb[:], in_=adjh)
            nc.sync.dma_start(out=hn_sb[:, 3:T], in_=hn[:, 3:T])

            nf = main.tile([B, T, D], f32)  # normalized node features
            invden = main.tile([B, T], f32)
            swdot = main.tile([B, NPAIR], f32)
            swe = main.tile([B, NPAIR], f32)  # exp(-dot/8)
            swp = main.tile([B, NPAIR], f32)  # 1 + exp(-dot/8)
            sw = main.tile([B, NPAIR], f32)  # sigmoid
            lnp = main.tile([B, NPAIR], f32)  # ln(1 + exp(-dot/8))
            enc = main.tile([B, NPAIR], f32)  # u8 code (pre-round)

            for t in range(T):
                plan = SCHED[t]
                # ---- keys^T for the NEXT step via PE transposes ----
                if t + 1 < T:
                    kt_ps = ps_kt.tile([D, C, 128], bf16, name="kt_ps")
                    for c in range(C):
                        nc.tensor.transpose(
                            kt_ps[:, c, 0:B], hn_sb[:, t + 1, c, 0:D], ident[0:B, 0:B]
                        )
                    kce = plan["kc_eng"]
                    eng = {"dve": nc.vector, "act": nc.scalar, "pool": nc.gpsimd}[kce]
                    if kce == "act":
                        nc.scalar.copy(keysT[:, t + 1], kt_ps[:, :, 0:B])
                    else:
                        eng.tensor_copy(keysT[:, t + 1], kt_ps[:, :, 0:B])

                # ---- transposed masked attention weights ----
                scT = ps_sc.tile([B, C, 128], f32, name="scT")
                for c in range(C):
                    nc.tensor.matmul(
                        scT[:, c, 0:B],
                        keysT[:, t, c, :],
                        qT_sb[:, t, :],
                        start=True,
                        stop=True,
                    )
                expsT = expp.tile([B, C, B], bf16, name="expsT")
                mexpT = mexpp.tile([B, C, B], bf16, name="mexpT")
                nhalf = 2 if t < 2 else 1
                for h in range(nhalf):
                    cs = slice(h * C // nhalf, (h + 1) * C // nhalf)
                    nc.scalar.activation(
                        expsT[:, cs], scT[:, cs, 0:B], Act.Exp, scale=0.125
                    )
                    nc.vector.tensor_tensor(
                        out=mexpT[:, cs], in0=expsT[:, cs], in1=adjT_sb[:, cs],
                        op=Alu.mult,
                    )
                # ---- node features + denominator (ones column) ----
                nf_ps = ps_nf.tile([B, 128], f32, name="nf_ps")
                for c in range(C):
                    nc.tensor.matmul(
                        nf_ps[:, 0:DP],
                        mexpT[:, c, :],
                        hn_sb[:, t, c, :],
                        start=(c == 0),
                        stop=(c == C - 1),
                    )
                nc.vector.reciprocal(invden[:, t : t + 1], nf_ps[:, D:DP])
                nc.vector.tensor_scalar_mul(
                    nf[:, t, :], nf_ps[:, 0:D], invden[:, t : t + 1]
                )

                # ---- sw pairs {t1 <= t}: sigmoid + u8 log code ----
                seg = t * (t + 1) // 2
                for t1 in range(t + 1):
                    prod = prodp.tile([B, D], f32, name="prod")
                    nc.vector.scalar_tensor_tensor(
                        out=prod[:],
                        in0=nf[:, t1, :],
                        scalar=1.0,
                        in1=nf[:, t, :],
                        op0=Alu.mult,
                        op1=Alu.mult,
                        accum_out=swdot[:, seg + t1 : seg + t1 + 1],
                    )
                pr = slice(seg, seg + t + 1)
                nc.scalar.activation(swe[:, pr], swdot[:, pr], Act.Exp, scale=-0.125)
                nc.scalar.activation(swp[:, pr], swe[:, pr], Act.Identity, bias=1.0)
                nc.vector.reciprocal(sw[:, pr], swp[:, pr])
                if U8_ENABLE:
                    nc.scalar.activation(lnp[:, pr], swp[:, pr], Act.Ln)
                    nc.vector.tensor_scalar(
                        out=enc[:, pr],
                        in0=lnp[:, pr],
                        scalar1=-AQ,
                        scalar2=BQ,
                        op0=Alu.mult,
                        op1=Alu.add,
                    )

                # ---- output blocks for step t ----
                kr, kc = plan["kr"], plan["kc"]
                u8_eng = list(plan["u8_eng"])

                def emit_u8(dst_ap, col):
                    e = u8_eng.pop(0)
                    if e == "act":
                        nc.scalar.activation(
                            dst_ap, adjh_sb[:], Act.Copy, scale=enc[:, col : col + 1]
                        )
                    elif e == "pool":
                        nc.gpsimd.tensor_scalar_mul(
                            dst_ap, adjh_sb[:], enc[:, col : col + 1]
                        )
                    else:
                        nc.vector.tensor_scalar_mul(
                            dst_ap, adjh_sb[:], enc[:, col : col + 1]
                        )

                # row tile s2 = t: f16 part t1 < kr, u8 part t1 >= kr
                if kr > 0:
                    r16 = r16p.tile([B, max(max_r16, 1), N], f16, name="r16")
                    for t1 in range(kr):
                        col = pidx(t1, t)
                        nc.vector.tensor_scalar_mul(
                            r16[:, t1, :], adjh_sb[:], sw[:, col : col + 1]
                        )
                    nc.sync.dma_start(
                        out=out16[t, :, 0 : kr * N],
                        in_=r16[:, 0:kr].rearrange("b t n -> b (t n)"),
                    )
                if t + 1 - kr > 0:
                    r8 = r8p.tile([B, max(max_r8, 1), N], u8, name="r8")
                    for t1 in range(kr, t + 1):
                        emit_u8(r8[:, t1 - kr, :], pidx(t1, t))
                    nc.sync.dma_start(
                        out=out8[t, :, kr * N : (t + 1) * N],
                        in_=r8[:, 0 : t + 1 - kr].rearrange("b t n -> b (t n)"),
                    )
                # col tile t1 = t: f16 part s2 < kc, u8 part s2 >= kc
                if kc > 0:
                    c16 = c16p.tile([B, max(max_c16, 1), N], f16, name="c16")
                    for s2 in range(kc):
                        col = pidx(t, s2)
                        nc.vector.tensor_scalar_mul(
                            c16[:, s2, :], adjh_sb[:], sw[:, col : col + 1]
                        )
                    nc.sync.dma_start(
                        out=out16_bsm[:, 0:kc, t * N : (t + 1) * N],
                        in_=c16[:, 0:kc],
                    )
                if t - kc > 0:
                    c8 = c8p.tile([B, max(max_c8, 1), N], u8, name="c8")
                    for s2 in range(kc, t):
                        emit_u8(c8[:, s2 - kc, :], pidx(t, s2))
                    nc.sync.dma_start(
                        out=out8_bsm[:, kc:t, t * N : (t + 1) * N],
                        in_=c8[:, 0 : t - kc],
                    )

    nc.compile()
    return nc


def _get_nc():
    if "nc" not in _CACHE:
        _CACHE["nc"] = _build()
    return _CACHE["nc"]


def make_in_maps(rf_f32, adj_i32):
    """Per-core input dicts from the full f32/i32 host arrays."""
    import ml_dtypes

    bf16 = ml_dtypes.bfloat16
    rf16 = np.asarray(rf_f32, dtype=np.float32).astype(bf16)  # [T, N, D]
    adji = np.asarray(adj_i32)
    rf_chunk = rf16.reshape(T, B, C, D)  # node = 8*j + c -> [t, j, c, d]
    hn = np.ones((B, T, C, DP), dtype=bf16)
    hn[..., 0:D] = rf_chunk.transpose(1, 0, 2, 3)
    hn = np.ascontiguousarray(hn)
    hT0 = np.ascontiguousarray(rf_chunk[0].transpose(2, 1, 0))  # [d, c, j]
    in_maps = []
    for k in range(NCORES):
        sl = slice(k * B, (k + 1) * B)
        qTk = np.ascontiguousarray(rf16[:, sl, :].transpose(2, 0, 1))  # [d, t, b]
        adjs = adji[sl, :]  # [b, m]
        adjTk = np.ascontiguousarray(
            adjs.reshape(B, B, C).transpose(1, 2, 0).astype(bf16)
        )  # [j, c, b]
        adjhk = np.ascontiguousarray(adjs.astype(np.float16))
        in_maps.append(
            {"qT": qTk, "hT0": hT0, "hn": hn, "adjT": adjTk, "adjh": adjhk}
        )
    return in_maps


def _is_u8_map():
    """[T(s2), T(t1)] bool: which blocks were written u8."""
    m = np.zeros((T, T), dtype=bool)
    for t in range(T):
        kr, kc = SCHED[t]["kr"], SCHED[t]["kc"]
        for t1 in range(t + 1):  # rows tile of step t: (s2=t, t1)
            m[t, t1] = t1 >= kr
        for s2 in range(t):  # cols tile of step t: (s2, t1=t)
            m[s2, t] = s2 >= kc
    return m


def kernel(raw_features, adj):
    from concourse.bass_utils import run_bass_kernel_spmd

    nc = _get_nc()
    in_maps = make_in_maps(raw_features, adj)
    res = run_bass_kernel_spmd(nc, in_maps, core_ids=list(range(NCORES)))
    lut = u8_lut()
    is_u8 = _is_u8_map()  # [s2, t1]
    out = np.empty((T * N, T * N), dtype=np.float32)
    ov = out.reshape(T, NCORES, B, T, N)
    for k in range(NCORES):
        o16 = np.asarray(res.results[k]["out16"]).reshape(T, B, T, N)
        o8 = np.asarray(res.results[k]["out8"]).reshape(T, B, T, N)
        dec = lut[o8]  # [s2, b, t1, a] f32
        f16v = o16.astype(np.float32)
        ov[:, k] = np.where(is_u8[:, None, :, None], dec, f16v)
    return out


# revision 7
# speedup vs baseline: 1.4703x; 1.1860x over previous
"""Trainium2 Bass kernel for nn_LocationEffect (GAT + temporal sigmoid attention).

out[s2*N+b, t1*N+a] = sw[b, t1, s2] * adj[b, a]
where sw = sigmoid(scale * nf nf^T per node), nf = GAT(raw_features, adj).

Sharding: row-shard the [12000, 12000] output over the node dim b.
Each of the 8 cores owns B = N/8 = 125 nodes: it computes the GAT for its
125 query rows (keys/values = all 1000 nodes, replicated), its sw slice
[125, 12, 12], and writes a [12, 125, 12000] output slab.

This version restructures the GAT so the per-step compute fits well under
the DMA roofline and mixes two output encodings to cut HBM write traffic:

- Scores are computed TRANSPOSED (scT[m, b] = k_m . q_b) so the masked
  exp weights come out of the Activation engine already in the [key-part,
  query] layout the nf matmul wants: no per-step PE transposes of the
  attention row and no PSUM->SBUF copy for it.
- The softmax denominator comes from a ones-column appended to h (column
  64 of each value chunk), so it falls out of the nf matmul for free.
- sigmoid is computed as 1/(1+exp(-x)) (Act exp + DVE reciprocal), and
  the u8 log-encode ln(sw) = -ln(1+exp(-x)) reuses the same intermediate:
  exp, ln, copy, identity all live in ONE act function table set
  (natural_log_exp_and_others), so the table is loaded exactly once.
- Output blocks are written either as f16 (2B) or as a log-encoded uint8
  (1B): q = round(BQ - AQ*ln(1/sw)); host decodes via a 256-entry LUT.
  Max elementwise rel err of the u8 encode is ~1.1e-2 and its norm-rel
  contribution ~6e-3, both far inside the 2e-2 gate. A small static
  scheduler picks, per GAT step, how many blocks go u8 vs f16 and which
  engine (DVE 4x-mode f16 = 321ns, Act copy-with-scale = 1018ns, Pool
  tensor-scalar = 1484ns, DVE u8 = 1102ns) computes each, balancing
  per-step engine busy time against the shrinking DMA time.
- All inputs are host-prepacked so each load is one HWDGE DMA with
  per-partition-contiguous descriptors (no SWDGE prep serialization).
"""

import math
import sys

import numpy as np

if "/opt/trn_rl_repo" not in sys.path:
    sys.path.insert(0, "/opt/trn_rl_repo")

T, N, D = 12, 1000, 64
NCORES = 8
B = N // NCORES  # 125 nodes per core
C = 8  # n-chunks of size B for K-tiling
DP = D + 1  # +1 ones column for the softmax denominator
NPAIR = T * (T + 1) // 2  # symmetric (t1 <= s2) pairs

U8_ENABLE = True
SW_FLOOR = 0.004  # smallest sw the u8 log code can represent
AQ = 254.0 / math.log(1.0 / SW_FLOOR)  # 46.005...
BQ = 255.0

_CACHE = {}


def pidx(a, b):
    """s2-major triangular index of the unordered pair {a, b}."""
    lo, hi = min(a, b), max(a, b)
    return hi * (hi + 1) // 2 + lo


def u8_lut():
    """Decode table: LUT[q] = sigmoid value encoded by code q (0 -> 0.0)."""
    q = np.arange(256, dtype=np.float64)
    lut = np.exp((q - BQ) / AQ)
    lut[0] = 0.0
    return lut.astype(np.float32)


def _schedule():
    """Per-step block plan.

    For GAT step t the kernel emits the new row tile (s2=t, t1=0..t) and
    the new column tile (t1=t, s2=0..t-1).  Each block is 125x1000 and can
    be written f16 (DVE fast mode) or u8 (any engine).  Choose per step:
      kr[t]: row blocks t1 <  kr are f16, t1 >= kr are u8
      kc[t]: col blocks s2 <  kc are f16, s2 >= kc are u8
    plus an engine for every u8 block and for the keysT PSUM->SBUF copy,
    minimizing the per-step makespan (engine busy vs DMA time).
    Returns list of dicts per t.
    """
    DVE_F16 = 0.321
    DVE_U8 = 0.581  # u8 out keeps DVE 2x_2p mode (all-SBUF), not 4x
    ACT_BLK = 1.018
    POOL_BLK = 1.484
    KC_COST = {"dve": 0.647, "act": 1.018, "pool": 1.484}
    sched = []
    for t in range(T):
        n = 2 * t + 1
        gat_dve = 1.08 + 0.13 * t
        gat_act = 1.60
        has_kc = t + 1 < T
        best = None
        for u in range(0, (n + 1) if U8_ENABLE else 1):
            f = n - u
            tdma = 0.694 * f + 0.347 * u
            for a in range(u + 1):
                for p in range(u - a + 1):
                    d = u - a - p
                    # Pool/GPSIMD cannot read PSUM, so the keysT copy is
                    # DVE or Act only.
                    for kce in ("dve", "act") if has_kc else ("none",):
                        tdve = gat_dve + DVE_F16 * f + DVE_U8 * d
                        tact = gat_act + ACT_BLK * a
                        tpool = POOL_BLK * p
                        if kce == "dve":
                            tdve += KC_COST["dve"]
                        elif kce == "act":
                            tact += KC_COST["act"]
                        elif kce == "pool":
                            tpool += KC_COST["pool"]
                        mk = max(tdve, tact, tpool, tdma)
                        # minimize makespan; tie-break: more u8 (less DMA),
                        # then lighter DVE (shorter critical path)
                        key = (round(mk * 16), -u, round(tdve * 16))
                        if best is None or key < best[0]:
                            best = (key, u, a, p, d, kce)
        _, u, a, p, d, kce = best
        f = n - u
        # distribute f16 blocks / u8 blocks over the concrete (row, col)
        # slots.  Rows tile has t+1 slots, cols tile t slots.  Put the f16
        # blocks at the low-t1 / low-s2 end (any split works; ranges must
        # be contiguous).  Rows first.
        fr = min(f, t + 1)
        fc = f - fr
        kr = fr  # rows t1 < kr f16
        kc = fc  # cols s2 < kc f16
        # engine list for u8 blocks in emission order (rows u8 then cols u8)
        engines = ["act"] * a + ["pool"] * p + ["dve"] * d
        # interleave so no engine gets all-consecutive blocks
        engines.sort()
        inter = []
        while engines:
            for e in ("act", "pool", "dve"):
                if e in engines:
                    engines.remove(e)
                    inter.append(e)
        sched.append(
            {"kr": kr, "kc": kc, "u8_eng": inter, "kc_eng": kce if has_kc else None}
        )
    return sched


SCHED = _schedule()


def _build():
    import concourse.bacc as bacc
    import concourse.mybir as mybir
    import concourse.tile as tile
    from concourse.masks import make_identity

    f32 = mybir.dt.float32
    f16 = mybir.dt.float16
    bf16 = mybir.dt.bfloat16
    u8 = mybir.dt.uint8
    Act = mybir.ActivationFunctionType
    Alu = mybir.AluOpType

    nc = bacc.Bacc(
        "TRN2",
        target_bir_lowering=False,
        debug=False,
        enable_asserts=False,
        num_devices=NCORES,
    )
    qT = nc.dram_tensor("qT", (D, T, B), bf16, kind="ExternalInput").ap()
    hT0 = nc.dram_tensor("hT0", (D, C, B), bf16, kind="ExternalInput").ap()
    hn = nc.dram_tensor("hn", (B, T, C, DP), bf16, kind="ExternalInput").ap()
    adjT = nc.dram_tensor("adjT", (B, C, B), bf16, kind="ExternalInput").ap()
    adjh = nc.dram_tensor("adjh", (B, N), f16, kind="ExternalInput").ap()
    out16 = nc.dram_tensor("out16", (T, B, T * N), f16, kind="ExternalOutput").ap()
    out16_bsm = out16.rearrange("s b m -> b s m")
    out8 = nc.dram_tensor("out8", (T, B, T * N), u8, kind="ExternalOutput").ap()
    out8_bsm = out8.rearrange("s b m -> b s m")

    # max tile sizes for the block pools, from the schedule
    max_r16 = max(SCHED[t]["kr"] for t in range(T))
    max_r8 = max(t + 1 - SCHED[t]["kr"] for t in range(T))
    max_c16 = max(SCHED[t]["kc"] for t in range(T))
    max_c8 = max(t - SCHED[t]["kc"] for t in range(T))

    with tile.TileContext(nc) as tc:
        # Pin ONE activation-function table set covering every func used
        # (exp, ln, identity, copy). Without this, the auto-inserted loads
        # thrash between exp-only and ln-only sets: 18 reloads x 1283ns.
        if U8_ENABLE:
            from concourse.hw_specs import get_activation_tables

            need = {Act.Exp, Act.Ln, Act.Identity, Act.Copy}
            set_id = None
            for i, (_, funcs) in enumerate(get_activation_tables(nc.m.arch).items()):
                if need <= funcs:
                    set_id = i
                    break
            assert set_id is not None, "no act table set covers exp+ln"
            ld = mybir.InstLoadActFuncSet(
                name=nc.get_next_instruction_name(), ins=[], outs=[]
            )
            ld.act_func_set_id = set_id
            nc.scalar.add_instruction(ld)

        with (
            tc.tile_pool(name="const", bufs=1) as consts,
            tc.tile_pool(name="main", bufs=1) as main,
            tc.tile_pool(name="expp", bufs=2) as expp,
            tc.tile_pool(name="mexpp", bufs=2) as mexpp,
            tc.tile_pool(name="prodp", bufs=2) as prodp,
            tc.tile_pool(name="r16p", bufs=2) as r16p,
            tc.tile_pool(name="r8p", bufs=2) as r8p,
            tc.tile_pool(name="c16p", bufs=2) as c16p,
            tc.tile_pool(name="c8p", bufs=2) as c8p,
            tc.tile_pool(name="ps_kt", bufs=2, space="PSUM") as ps_kt,
            tc.tile_pool(name="ps_sc", bufs=2, space="PSUM") as ps_sc,
            tc.tile_pool(name="ps_nf", bufs=2, space="PSUM") as ps_nf,
        ):
            ident = consts.tile([128, 128], bf16)
            make_identity(nc, ident[:])

            # ---- inputs: critical pieces first, all single HWDGE DMAs ----
            qT_sb = main.tile([D, T, B], bf16)
            nc.sync.dma_start(out=qT_sb[:], in_=qT)
            keysT = main.tile([D, T, C, B], bf16)
            nc.sync.dma_start(out=keysT[:, 0], in_=hT0)
            adjT_sb = main.tile([B, C, B], bf16)
            nc.sync.dma_start(out=adjT_sb[:], in_=adjT)
            hn_sb = main.tile([B, T, C, DP], bf16)
            nc.sync.dma_start(out=hn_sb[:, 0:3], in_=hn[:, 0:3])
            adjh_sb = main.tile([B, N], f16)
            nc.sync.dma_start(out=adjh_sb[:], in_=adjh)
            nc.sync.dma_start(out=hn_sb[:, 3:T], in_=hn[:, 3:T])

            nf = main.tile([B, T, D], f32)  # normalized node features
            invden = main.tile([B, T], f32)
            swdot = main.tile([B, NPAIR], f32)
            swe = main.tile([B, NPAIR], f32)  # exp(-dot/8)
            swp = main.tile([B, NPAIR], f32)  # 1 + exp(-dot/8)
            sw = main.tile([B, NPAIR], f32)  # sigmoid
            lnp = main.tile([B, NPAIR], f32)  # ln(1 + exp(-dot/8))
            enc = main.tile([B, NPAIR], f32)  # u8 code (pre-round)

            for t in range(T):
                plan = SCHED[t]
                # ---- keys^T for the NEXT step via PE transposes ----
                if t + 1 < T:
                    kt_ps = ps_kt.tile([D, C, 128], bf16, name="kt_ps")
                    for c in range(C):
                        nc.tensor.transpose(
                            kt_ps[:, c, 0:B], hn_sb[:, t + 1, c, 0:D], ident[0:B, 0:B]
                        )
                    kce = plan["kc_eng"]
                    eng = {"dve": nc.vector, "act": nc.scalar, "pool": nc.gpsimd}[kce]
                    if kce == "act":
                        nc.scalar.copy(keysT[:, t + 1], kt_ps[:, :, 0:B])
                    else:
                        eng.tensor_copy(keysT[:, t + 1], kt_ps[:, :, 0:B])

                # ---- transposed masked attention weights ----
                scT = ps_sc.tile([B, C, 128], f32, name="scT")
                for c in range(C):
                    nc.tensor.matmul(
                        scT[:, c, 0:B],
                        keysT[:, t, c, :],
                        qT_sb[:, t, :],
                        start=True,
                        stop=True,
                    )
                expsT = expp.tile([B, C, B], bf16, name="expsT")
                mexpT = mexpp.tile([B, C, B], bf16, name="mexpT")
                nhalf = 2 if t < 2 else 1
                for h in range(nhalf):
                    cs = slice(h * C // nhalf, (h + 1) * C // nhalf)
                    nc.scalar.activation(
                        expsT[:, cs], scT[:, cs, 0:B], Act.Exp, scale=0.125
                    )
                    nc.vector.tensor_tensor(
                        out=mexpT[:, cs], in0=expsT[:, cs], in1=adjT_sb[:, cs],
                        op=Alu.mult,
                    )
                # ---- node features + denominator (ones column) ----
                nf_ps = ps_nf.tile([B, 128], f32, name="nf_ps")
                for c in range(C):
                    nc.tensor.matmul(
                        nf_ps[:, 0:DP],
                        mexpT[:, c, :],
                        hn_sb[:, t, c, :],
                        start=(c == 0),
                        stop=(c == C - 1),
                    )
                nc.vector.reciprocal(invden[:, t : t + 1], nf_ps[:, D:DP])
                nc.vector.tensor_scalar_mul(
                    nf[:, t, :], nf_ps[:, 0:D], invden[:, t : t + 1]
                )

                # ---- sw pairs {t1 <= t}: sigmoid + u8 log code ----
                seg = t * (t + 1) // 2
                for t1 in range(t + 1):
                    prod = prodp.tile([B, D], f32, name="prod")
                    nc.vector.scalar_tensor_tensor(
                        out=prod[:],
                        in0=nf[:, t1, :],
                        scalar=1.0,
                        in1=nf[:, t, :],
                        op0=Alu.mult,
                        op1=Alu.mult,
                        accum_out=swdot[:, seg + t1 : seg + t1 + 1],
                    )
                pr = slice(seg, seg + t + 1)
                nc.scalar.activation(swe[:, pr], swdot[:, pr], Act.Exp, scale=-0.125)
                nc.scalar.activation(swp[:, pr], swe[:, pr], Act.Identity, bias=1.0)
                nc.vector.reciprocal(sw[:, pr], swp[:, pr])
                if U8_ENABLE:
                    nc.scalar.activation(lnp[:, pr], swp[:, pr], Act.Ln)
                    nc.vector.tensor_scalar(
                        out=enc[:, pr],
                        in0=lnp[:, pr],
                        scalar1=-AQ,
                        scalar2=BQ,
                        op0=Alu.mult,
                        op1=Alu.add,
                    )

                # ---- output blocks for step t ----
                kr, kc = plan["kr"], plan["kc"]
                u8_eng = list(plan["u8_eng"])

                def emit_u8(dst_ap, col):
                    e = u8_eng.pop(0)
                    if e == "act":
                        nc.scalar.activation(
                            dst_ap, adjh_sb[:], Act.Copy, scale=enc[:, col : col + 1]
                        )
                    elif e == "pool":
                        nc.gpsimd.tensor_scalar_mul(
                            dst_ap, adjh_sb[:], enc[:, col : col + 1]
                        )
                    else:
                        nc.vector.tensor_scalar_mul(
                            dst_ap, adjh_sb[:], enc[:, col : col + 1]
                        )

                # row tile s2 = t: f16 part t1 < kr, u8 part t1 >= kr
                if kr > 0:
                    r16 = r16p.tile([B, max(max_r16, 1), N], f16, name="r16")
                    for t1 in range(kr):
                        col = pidx(t1, t)
                        nc.vector.tensor_scalar_mul(
                            r16[:, t1, :], adjh_sb[:], sw[:, col : col + 1]
                        )
                    nc.sync.dma_start(
                        out=out16[t, :, 0 : kr * N],
                        in_=r16[:, 0:kr].rearrange("b t n -> b (t n)"),
                    )
                if t + 1 - kr > 0:
                    r8 = r8p.tile([B, max(max_r8, 1), N], u8, name="r8")
                    for t1 in range(kr, t + 1):
                        emit_u8(r8[:, t1 - kr, :], pidx(t1, t))
                    nc.sync.dma_start(
                        out=out8[t, :, kr * N : (t + 1) * N],
                        in_=r8[:, 0 : t + 1 - kr].rearrange("b t n -> b (t n)"),
                    )
                # col tile t1 = t: f16 part s2 < kc, u8 part s2 >= kc
                if kc > 0:
                    c16 = c16p.tile([B, max(max_c16, 1), N], f16, name="c16")
                    for s2 in range(kc):
                        col = pidx(t, s2)
                        nc.vector.tensor_scalar_mul(
                            c16[:, s2, :], adjh_sb[:], sw[:, col : col + 1]
                        )
                    nc.sync.dma_start(
                        out=out16_bsm[:, 0:kc, t * N : (t + 1) * N],
                        in_=c16[:, 0:kc],
                    )
                if t - kc > 0:
                    c8 = c8p.tile([B, max(max_c8, 1), N], u8, name="c8")
                    for s2 in range(kc, t):
                        emit_u8(c8[:, s2 - kc, :], pidx(t, s2))
                    nc.sync.dma_start(
                        out=out8_bsm[:, kc:t, t * N : (t + 1) * N],
                        in_=c8[:, 0 : t - kc],
                    )

    nc.compile()
    return nc


def _get_nc():
    if "nc" not in _CACHE:
        _CACHE["nc"] = _build()
    return _CACHE["nc"]


def make_in_maps(rf_f32, adj_i32):
    """Per-core input dicts from the full f32/i32 host arrays."""
    import ml_dtypes

    bf16 = ml_dtypes.bfloat16
    rf16 = np.asarray(rf_f32, dtype=np.float32).astype(bf16)  # [T, N, D]
    adji = np.asarray(adj_i32)
    rf_chunk = rf16.reshape(T, B, C, D)  # node = 8*j + c -> [t, j, c, d]
    hn = np.ones((B, T, C, DP), dtype=bf16)
    hn[..., 0:D] = rf_chunk.transpose(1, 0, 2, 3)
    hn = np.ascontiguousarray(hn)
    hT0 = np.ascontiguousarray(rf_chunk[0].transpose(2, 1, 0))  # [d, c, j]
    in_maps = []
    for k in range(NCORES):
        sl = slice(k * B, (k + 1) * B)
        qTk = np.ascontiguousarray(rf16[:, sl, :].transpose(2, 0, 1))  # [d, t, b]
        adjs = adji[sl, :]  # [b, m]
        adjTk = np.ascontiguousarray(
            adjs.reshape(B, B, C).transpose(1, 2, 0).astype(bf16)
        )  # [j, c, b]
        adjhk = np.ascontiguousarray(adjs.astype(np.float16))
        in_maps.append(
            {"qT": qTk, "hT0": hT0, "hn": hn, "adjT": adjTk, "adjh": adjhk}
        )
    return in_maps


def _is_u8_map():
    """[T(s2), T(t1)] bool: which blocks were written u8."""
    m = np.zeros((T, T), dtype=bool)
    for t in range(T):
        kr, kc = SCHED[t]["kr"], SCHED[t]["kc"]
        for t1 in range(t + 1):  # rows tile of step t: (s2=t, t1)
            m[t, t1] = t1 >= kr
        for s2 in range(t):  # cols tile of step t: (s2, t1=t)
            m[s2, t] = s2 >= kc
    return m


def kernel(raw_features, adj):
    from concourse.bass_utils import run_bass_kernel_spmd

    nc = _get_nc()
    in_maps = make_in_maps(raw_features, adj)
    res = run_bass_kernel_spmd(nc, in_maps, core_ids=list(range(NCORES)))
    lut = u8_lut()
    is_u8 = _is_u8_map()  # [s2, t1]
    out = np.empty((T * N, T * N), dtype=np.float32)
    ov = out.reshape(T, NCORES, B, T, N)
    for k in range(NCORES):
        o16 = np.asarray(res.results[k]["out16"]).reshape(T, B, T, N)
        o8 = np.asarray(res.results[k]["out8"]).reshape(T, B, T, N)
        dec = lut[o8]  # [s2, b, t1, a] f32
        f16v = o16.astype(np.float32)
        ov[:, k] = np.where(is_u8[:, None, :, None], dec, f16v)
    return out


# revision 31
# speedup vs baseline: 1.5991x; 1.0876x over previous
"""Trainium2 Bass kernel for nn_LocationEffect (GAT + temporal sigmoid attention).

out[s2*N+b, t1*N+a] = sw[b, t1, s2] * adj[b, a]
where sw = sigmoid(scale * nf nf^T per node), nf = GAT(raw_features, adj).

Sharding: row-shard the [12000, 12000] output over the node dim b.
Each of the 8 cores owns B = N/8 = 125 nodes: it computes the GAT for its
125 query rows (keys/values = all 1000 nodes, replicated), its sw slice
[125, 12, 12], and writes a [12, 125, 12000] output slab.

This version restructures the GAT so the per-step compute fits well under
the DMA roofline and mixes two output encodings to cut HBM write traffic:

- Scores are computed TRANSPOSED (scT[m, b] = k_m . q_b) so the masked
  exp weights come out of the Activation engine already in the [key-part,
  query] layout the nf matmul wants: no per-step PE transposes of the
  attention row and no PSUM->SBUF copy for it.
- The softmax denominator comes from a ones-column appended to h (column
  64 of each value chunk), so it falls out of the nf matmul for free.
- sigmoid is computed as 1/(1+exp(-x)) (Act exp + DVE reciprocal), and
  the u8 log-encode ln(sw) = -ln(1+exp(-x)) reuses the same intermediate:
  exp, ln, copy, identity all live in ONE act function table set
  (natural_log_exp_and_others), so the table is loaded exactly once.
- Output blocks are written either as f16 (2B) or as a log-encoded uint8
  (1B): q = round(BQ - AQ*ln(1/sw)); host decodes via a 256-entry LUT.
  Max elementwise rel err of the u8 encode is ~1.1e-2 and its norm-rel
  contribution ~6e-3, both far inside the 2e-2 gate. A small static
  scheduler picks, per GAT step, how many blocks go u8 vs f16 and which
  engine (DVE 4x-mode f16 = 321ns, Act copy-with-scale = 1018ns, Pool
  tensor-scalar = 1484ns, DVE u8 = 1102ns) computes each, balancing
  per-step engine busy time against the shrinking DMA time.
- All inputs are host-prepacked so each load is one HWDGE DMA with
  per-partition-contiguous descriptors (no SWDGE prep serialization).
"""

import math
import sys

import numpy as np

if "/opt/trn_rl_repo" not in sys.path:
    sys.path.insert(0, "/opt/trn_rl_repo")

T, N, D = 12, 1000, 64
NCORES = 8
B = N // NCORES  # 125 nodes per core
C = 8  # n-chunks of size B for K-tiling
DP = D + 1  # +1 ones column for the softmax denominator
NPAIR = T * (T + 1) // 2  # symmetric (t1 <= s2) pairs

U8_ENABLE = True
SW_FLOOR = 0.004  # smallest sw the u8 log code can represent
AQ = 254.0 / math.log(1.0 / SW_FLOOR)  # 46.005...
BQ = 255.0

_CACHE = {}


def pidx(a, b):
    """s2-major triangular index of the unordered pair {a, b}."""
    lo, hi = min(a, b), max(a, b)
    return hi * (hi + 1) // 2 + lo


def u8_lut():
    """Decode table: LUT[q] = sigmoid value encoded by code q (0 -> 0.0)."""
    q = np.arange(256, dtype=np.float64)
    lut = np.exp((q - BQ) / AQ)
    lut[0] = 0.0
    return lut.astype(np.float32)


def _schedule():
    """Per-step block plan.

    For GAT step t the kernel emits the new row tile (s2=t, t1=0..t) and
    the new column tile (t1=t, s2=0..t-1).  Each block is 125x1000 and can
    be written f16 (DVE fast mode) or u8 (any engine).  Choose per step:
      kr[t]: row blocks t1 <  kr are f16, t1 >= kr are u8
      kc[t]: col blocks s2 <  kc are f16, s2 >= kc are u8
    plus an engine for every u8 block and for the keysT PSUM->SBUF copy,
    minimizing the per-step makespan (engine busy vs DMA time).
    Returns list of dicts per t.
    """
    DVE_F16 = 0.321
    DVE_U8 = 0.581  # u8 out keeps DVE 2x_2p mode (all-SBUF), not 4x
    ACT_BLK = 1.018
    POOL_BLK = 1.484
    sched = []
    for t in range(T):
        n = 2 * t + 1
        gat_dve = 1.10 + 0.13 * t
        gat_act = 1.40
        best = None
        for u in range(0, (n + 1) if U8_ENABLE else 1):
            f = n - u
            tdma = 0.694 * f + 0.347 * u
            for a in range(u + 1):
                for p in range(u - a + 1):
                    d = u - a - p
                    tdve = gat_dve + DVE_F16 * f + DVE_U8 * d
                    tact = gat_act + ACT_BLK * a
                    tpool = POOL_BLK * p
                    mk = max(tdve, tact, tpool, tdma)
                    # minimize makespan; tie-break: more u8 (less DMA),
                    # then lighter DVE (shorter critical path)
                    key = (round(mk * 16), -u, round(tdve * 16))
                    if best is None or key < best[0]:
                        best = (key, u, a, p, d)
        _, u, a, p, d = best
        kce = "dma"  # keysT PSUM->SBUF moves by DMA: zero engine time
        f = n - u
        # distribute f16 blocks / u8 blocks over the concrete (row, col)
        # slots.  Rows tile has t+1 slots, cols tile t slots.  Put the f16
        # blocks at the low-t1 / low-s2 end (any split works; ranges must
        # be contiguous).  Rows first.
        fr = min(f, t + 1)
        fc = f - fr
        kr = fr  # rows t1 < kr f16
        kc = fc  # cols s2 < kc f16
        # engine list for u8 blocks in emission order (rows u8 then cols u8)
        engines = ["act"] * a + ["pool"] * p + ["dve"] * d
        # interleave so no engine gets all-consecutive blocks
        engines.sort()
        inter = []
        while engines:
            for e in ("act", "pool", "dve"):
                if e in engines:
                    engines.remove(e)
                    inter.append(e)
        sched.append({"kr": kr, "kc": kc, "u8_eng": inter, "kc_eng": kce})
    # Latency override: the first steps sit on the critical path to the
    # first output DMA while the DMA engines are still busy with inputs —
    # f16 blocks on DVE (321ns) beat any u8 engine there.
    for t in range(3):
        n = 2 * t + 1
        sched[t]["kr"] = t + 1
        sched[t]["kc"] = t
        sched[t]["u8_eng"] = []
    return sched


SCHED = _schedule()


def _build():
    import concourse.bacc as bacc
    import concourse.mybir as mybir
    import concourse.tile as tile
    from concourse.masks import make_identity

    f32 = mybir.dt.float32
    f16 = mybir.dt.float16
    bf16 = mybir.dt.bfloat16
    u8 = mybir.dt.uint8
    Act = mybir.ActivationFunctionType
    Alu = mybir.AluOpType

    nc = bacc.Bacc(
        "TRN2",
        target_bir_lowering=False,
        debug=False,
        enable_asserts=False,
        num_devices=NCORES,
    )
    qT = nc.dram_tensor("qT", (D, T, B), bf16, kind="ExternalInput").ap()
    hT0 = nc.dram_tensor("hT0", (D, T, C, B), bf16, kind="ExternalInput").ap()
    hn = nc.dram_tensor("hn", (B, T, C, DP), bf16, kind="ExternalInput").ap()
    adjT = nc.dram_tensor("adjT", (B, C, B), bf16, kind="ExternalInput").ap()
    adjh = nc.dram_tensor("adjh", (B, N), f16, kind="ExternalInput").ap()
    out16 = nc.dram_tensor("out16", (T, B, T * N), f16, kind="ExternalOutput").ap()
    out16_bsm = out16.rearrange("s b m -> b s m")
    out8 = nc.dram_tensor("out8", (T, B, T * N), u8, kind="ExternalOutput").ap()
    out8_bsm = out8.rearrange("s b m -> b s m")

    # max tile sizes for the block pools, from the schedule
    max_r16 = max(SCHED[t]["kr"] for t in range(T))
    max_r8 = max(t + 1 - SCHED[t]["kr"] for t in range(T))
    max_c16 = max(SCHED[t]["kc"] for t in range(T))
    max_c8 = max(t - SCHED[t]["kc"] for t in range(T))

    with tile.TileContext(nc) as tc:
        # Pin ONE activation-function table set covering every func used
        # (exp, ln, identity, copy). Without this, the auto-inserted loads
        # thrash between exp-only and ln-only sets: 18 reloads x 1283ns.
        if U8_ENABLE:
            from concourse.hw_specs import get_activation_tables

            need = {Act.Exp, Act.Ln, Act.Identity, Act.Copy}
            set_id = None
            for i, (_, funcs) in enumerate(get_activation_tables(nc.m.arch).items()):
                if need <= funcs:
                    set_id = i
                    break
            assert set_id is not None, "no act table set covers exp+ln"
            ld = mybir.InstLoadActFuncSet(
                name=nc.get_next_instruction_name(), ins=[], outs=[]
            )
            ld.act_func_set_id = set_id
            nc.scalar.add_instruction(ld)

        with (
            tc.tile_pool(name="const", bufs=1) as consts,
            tc.tile_pool(name="main", bufs=1) as main,
            tc.tile_pool(name="expp", bufs=2) as expp,
            tc.tile_pool(name="mexpp", bufs=2) as mexpp,
            tc.tile_pool(name="prodp", bufs=2) as prodp,
            tc.tile_pool(name="r16p", bufs=2) as r16p,
            tc.tile_pool(name="r8p", bufs=2) as r8p,
            tc.tile_pool(name="c16p", bufs=2) as c16p,
            tc.tile_pool(name="c8p", bufs=2) as c8p,
            tc.tile_pool(name="ps_sc", bufs=2, space="PSUM") as ps_sc,
            tc.tile_pool(name="ps_nf", bufs=2, space="PSUM") as ps_nf,
        ):
            ident = consts.tile([128, 128], bf16)
            make_identity(nc, ident[:])
            # PE p-state warmup: tiny transposes keep the PE "busy" early so
            # the real t0 matmuls run at mid/full clock instead of 0.65 GHz.
            warm_ps = ps_nf.tile([B, 128], bf16, name="warm")
            for w in range(10):
                nc.tensor.transpose(
                    warm_ps[0:64, 0:64], ident[0:64, 0:64], ident[0:64, 0:64]
                )

            # ---- inputs: ordered by first use on the t0 critical chain ----
            qT_sb = main.tile([D, T, B], bf16)
            nc.sync.dma_start(out=qT_sb[:], in_=qT)
            keysT = main.tile([D, T, C, B], bf16)
            nc.sync.dma_start(out=keysT[:, 0:1], in_=hT0[:, 0:1])
            adjT_sb = main.tile([B, C, B], bf16)
            nc.sync.dma_start(out=adjT_sb[:], in_=adjT)
            hn_sb = main.tile([B, T, C, DP], bf16)
            nc.sync.dma_start(out=hn_sb[:, 0:1], in_=hn[:, 0:1])
            adjh_sb = main.tile([B, N], f16)
            nc.sync.dma_start(out=adjh_sb[:], in_=adjh)
            nc.sync.dma_start(out=keysT[:, 1:3], in_=hT0[:, 1:3])
            nc.sync.dma_start(out=hn_sb[:, 1:3], in_=hn[:, 1:3])
            nc.sync.dma_start(out=keysT[:, 3:6], in_=hT0[:, 3:6])
            nc.sync.dma_start(out=hn_sb[:, 3:6], in_=hn[:, 3:6])
            nc.sync.dma_start(out=keysT[:, 6:T], in_=hT0[:, 6:T])
            nc.sync.dma_start(out=hn_sb[:, 6:T], in_=hn[:, 6:T])

            nf = main.tile([B, T, D], f32)  # normalized node features
            invden = main.tile([B, T], f32)
            swdot = main.tile([B, NPAIR], f32)
            swe = main.tile([B, NPAIR], f32)  # exp(-dot/8)
            swp = main.tile([B, NPAIR], f32)  # 1 + exp(-dot/8)
            sw = main.tile([B, NPAIR], f32)  # sigmoid
            lnp = main.tile([B, NPAIR], f32)  # ln(1 + exp(-dot/8))
            enc = main.tile([B, NPAIR], f32)  # u8 code (pre-round)

            for t in range(T):
                plan = SCHED[t]
                # ---- transposed masked attention weights ----
                scT = ps_sc.tile([B, C, 128], f32, name="scT")
                for c in range(C):
                    nc.tensor.matmul(
                        scT[:, c, 0:B],
                        keysT[:, t, c, :],
                        qT_sb[:, t, :],
                        start=True,
                        stop=True,
                    )
                expsT = expp.tile([B, C, B], bf16, name="expsT")
                mexpT = mexpp.tile([B, C, B], bf16, name="mexpT")
                for h in range(2):
                    cs = slice(h * C // 2, (h + 1) * C // 2)
                    nc.scalar.activation(
                        expsT[:, cs], scT[:, cs, 0:B], Act.Exp, scale=0.125
                    )
                    nc.vector.tensor_tensor(
                        out=mexpT[:, cs], in0=expsT[:, cs], in1=adjT_sb[:, cs],
                        op=Alu.mult,
                    )
                # ---- node features + denominator (ones column) ----
                nf_ps = ps_nf.tile([B, 128], f32, name="nf_ps")
                for c in range(C):
                    nc.tensor.matmul(
                        nf_ps[:, 0:DP],
                        mexpT[:, c, :],
                        hn_sb[:, t, c, :],
                        start=(c == 0),
                        stop=(c == C - 1),
                    )
                nc.vector.reciprocal(invden[:, t : t + 1], nf_ps[:, D:DP])
                nc.vector.tensor_scalar_mul(
                    nf[:, t, :], nf_ps[:, 0:D], invden[:, t : t + 1]
                )

                # ---- sw pairs {t1 <= t}: sigmoid + u8 log code ----
                seg = t * (t + 1) // 2
                for t1 in range(t + 1):
                    prod = prodp.tile([B, D], f32, name="prod")
                    nc.vector.scalar_tensor_tensor(
                        out=prod[:],
                        in0=nf[:, t1, :],
                        scalar=1.0,
                        in1=nf[:, t, :],
                        op0=Alu.mult,
                        op1=Alu.mult,
                        accum_out=swdot[:, seg + t1 : seg + t1 + 1],
                    )
                pr = slice(seg, seg + t + 1)
                nc.scalar.activation(swe[:, pr], swdot[:, pr], Act.Exp, scale=-0.125)
                nc.vector.tensor_scalar_add(swp[:, pr], swe[:, pr], 1.0)
                nc.vector.reciprocal(sw[:, pr], swp[:, pr])
                if U8_ENABLE:
                    # ln(1 + e^{-x}) in one op via the activation bias
                    nc.scalar.activation(lnp[:, pr], swe[:, pr], Act.Ln, bias=1.0)
                    nc.vector.tensor_scalar(
                        out=enc[:, pr],
                        in0=lnp[:, pr],
                        scalar1=-AQ,
                        scalar2=BQ,
                        op0=Alu.mult,
                        op1=Alu.add,
                    )

                # ---- output blocks for step t ----
                kr, kc = plan["kr"], plan["kc"]
                u8_eng = list(plan["u8_eng"])

                def emit_u8(dst_ap, col):
                    e = u8_eng.pop(0)
                    if e == "act":
                        nc.scalar.activation(
                            dst_ap, adjh_sb[:], Act.Copy, scale=enc[:, col : col + 1]
                        )
                    elif e == "pool":
                        nc.gpsimd.tensor_scalar_mul(
                            dst_ap, adjh_sb[:], enc[:, col : col + 1]
                        )
                    else:
                        nc.vector.tensor_scalar_mul(
                            dst_ap, adjh_sb[:], enc[:, col : col + 1]
                        )

                # row tile s2 = t: f16 part t1 < kr, u8 part t1 >= kr
                if kr > 0:
                    r16 = r16p.tile([B, max(max_r16, 1), N], f16, name="r16")
                    for t1 in range(kr):
                        col = pidx(t1, t)
                        nc.vector.tensor_scalar_mul(
                            r16[:, t1, :], adjh_sb[:], sw[:, col : col + 1]
                        )
                    nc.sync.dma_start(
                        out=out16[t, :, 0 : kr * N],
                        in_=r16[:, 0:kr].rearrange("b t n -> b (t n)"),
                    )
                if t + 1 - kr > 0:
                    r8 = r8p.tile([B, max(max_r8, 1), N], u8, name="r8")
                    nu = t + 1 - kr
                    # split big tiles into two DMAs so the first half ships
                    # while the second half is still being computed
                    cuts = [0, nu // 2, nu] if nu >= 6 else [0, nu]
                    for lo, hi in zip(cuts, cuts[1:]):
                        for t1 in range(kr + lo, kr + hi):
                            emit_u8(r8[:, t1 - kr, :], pidx(t1, t))
                        nc.sync.dma_start(
                            out=out8[t, :, (kr + lo) * N : (kr + hi) * N],
                            in_=r8[:, lo:hi].rearrange("b t n -> b (t n)"),
                        )
                # col tile t1 = t: f16 part s2 < kc, u8 part s2 >= kc
                if kc > 0:
                    c16 = c16p.tile([B, max(max_c16, 1), N], f16, name="c16")
                    for s2 in range(kc):
                        col = pidx(t, s2)
                        nc.vector.tensor_scalar_mul(
                            c16[:, s2, :], adjh_sb[:], sw[:, col : col + 1]
                        )
                    nc.sync.dma_start(
                        out=out16_bsm[:, 0:kc, t * N : (t + 1) * N],
                        in_=c16[:, 0:kc],
                    )
                if t - kc > 0:
                    c8 = c8p.tile([B, max(max_c8, 1), N], u8, name="c8")
                    nu = t - kc
                    cuts = [0, nu // 2, nu] if nu >= 6 else [0, nu]
                    for lo, hi in zip(cuts, cuts[1:]):
                        for s2 in range(kc + lo, kc + hi):
                            emit_u8(c8[:, s2 - kc, :], pidx(t, s2))
                        nc.sync.dma_start(
                            out=out8_bsm[:, kc + lo : kc + hi, t * N : (t + 1) * N],
                            in_=c8[:, lo:hi],
                        )



    nc.compile()
    return nc


def _get_nc():
    if "nc" not in _CACHE:
        _CACHE["nc"] = _build()
    return _CACHE["nc"]


def make_in_maps(rf_f32, adj_i32):
    """Per-core input dicts from the full f32/i32 host arrays."""
    import ml_dtypes

    bf16 = ml_dtypes.bfloat16
    rf16 = np.asarray(rf_f32, dtype=np.float32).astype(bf16)  # [T, N, D]
    adji = np.asarray(adj_i32)
    rf_chunk = rf16.reshape(T, B, C, D)  # node = 8*j + c -> [t, j, c, d]
    hn = np.ones((B, T, C, DP), dtype=bf16)
    hn[..., 0:D] = rf_chunk.transpose(1, 0, 2, 3)
    hn = np.ascontiguousarray(hn)
    hT0 = np.ascontiguousarray(rf_chunk.transpose(3, 0, 2, 1))  # [d, t, c, j]
    in_maps = []
    for k in range(NCORES):
        sl = slice(k * B, (k + 1) * B)
        qTk = np.ascontiguousarray(rf16[:, sl, :].transpose(2, 0, 1))  # [d, t, b]
        adjs = adji[sl, :]  # [b, m]
        adjTk = np.ascontiguousarray(
            adjs.reshape(B, B, C).transpose(1, 2, 0).astype(bf16)
        )  # [j, c, b]
        adjhk = np.ascontiguousarray(adjs.astype(np.float16))
        in_maps.append(
            {"qT": qTk, "hT0": hT0, "hn": hn, "adjT": adjTk, "adjh": adjhk}
        )
    return in_maps


def _is_u8_map():
    """[T(s2), T(t1)] bool: which blocks were written u8."""
    m = np.zeros((T, T), dtype=bool)
    for t in range(T):
        kr, kc = SCHED[t]["kr"], SCHED[t]["kc"]
        for t1 in range(t + 1):  # rows tile of step t: (s2=t, t1)
            m[t, t1] = t1 >= kr
        for s2 in range(t):  # cols tile of step t: (s2, t1=t)
            m[s2, t] = s2 >= kc
    return m


def kernel(raw_features, adj):
    from concourse.bass_utils import run_bass_kernel_spmd

    nc = _get_nc()
    in_maps = make_in_maps(raw_features, adj)
    res = run_bass_kernel_spmd(nc, in_maps, core_ids=list(range(NCORES)))
    lut = u8_lut()
    is_u8 = _is_u8_map()  # [s2, t1]
    out = np.empty((T * N, T * N), dtype=np.float32)
    ov = out.reshape(T, NCORES, B, T, N)
    for k in range(NCORES):
        o16 = np.asarray(res.results[k]["out16"]).reshape(T, B, T, N)
        o8 = np.asarray(res.results[k]["out8"]).reshape(T, B, T, N)
        dec = lut[o8]  # [s2, b, t1, a] f32
        f16v = o16.astype(np.float32)
        ov[:, k] = np.where(is_u8[:, None, :, None], dec, f16v)
    return out
